# revision 9
# baseline (speedup 1.0000x reference)
"""Trainium2 Bass kernel for CCTAttention (GQA attention + RoPE + precision-bias gain).

Sharding: 8 cores = (batch b in {0,1}) x (kv-group g in {0..3}).
Core c = 4*b + g computes q-heads 4g..4g+3 and kv head g for batch b:
  - projections (q/k/v) from hs[b] in transposed [d, s] layout
  - RoPE via partition-shift copies
  - scores = (q.k)*SCALE*(1+pbias) + mask, softmax (full S x S, written out)
  - out_partial = attn @ v @ wo_shard.T  (host sums the 4 partials per batch)
"""

import numpy as np

B, S, HID = 2, 2048, 2048
NH, NKV, HD = 16, 4, 128
G = NH // NKV          # q-heads per core
SCALE = HD ** -0.5

_CACHE = {}


def _build():
    import concourse.bacc as bacc
    import concourse.tile as tile
    from concourse import mybir
    from concourse.masks import make_identity

    f32 = mybir.dt.float32
    f32r = mybir.dt.float32r
    bf16 = mybir.dt.bfloat16
    Alu = mybir.AluOpType
    Act = mybir.ActivationFunctionType

    nc = bacc.Bacc()

    hsT_d = nc.declare_dram_parameter("hsT", [HID, S], f32, isOutput=False)
    wqT_d = nc.declare_dram_parameter("wqT", [HID, G * HD], f32, isOutput=False)
    wkT_d = nc.declare_dram_parameter("wkT", [HID, HD], f32, isOutput=False)
    wvT_d = nc.declare_dram_parameter("wvT", [HID, HD], f32, isOutput=False)
    woT_d = nc.declare_dram_parameter("woT", [G * HD, HID], f32, isOutput=False)
    cosT_d = nc.declare_dram_parameter("cosT", [HD, S], f32, isOutput=False)
    sinTs_d = nc.declare_dram_parameter("sinTs", [HD, S], f32, isOutput=False)
    mask_d = nc.declare_dram_parameter("mask", [S, S], f32, isOutput=False)
    gain_d = nc.declare_dram_parameter("gain", [128, 16], f32, isOutput=False)
    attn_d = nc.declare_dram_parameter("attn_out", [G, S, S], f32, isOutput=True)
    outp_d = nc.declare_dram_parameter("out_partial", [S, HID], f32, isOutput=True)

    KK = HID // 128     # 16 contraction tiles
    NQC = 8             # q-chunks of 256
    QCW = 256

    with tile.TileContext(nc) as tc, tc.tile_pool(name="persist", bufs=1) as pp:
        ident = pp.tile([128, 128], f32, tag="ident")
        make_identity(nc, ident)
        gain_sb = pp.tile([128, 16], f32, tag="gain")
        nc.sync.dma_start(out=gain_sb, in_=gain_d[:, :])
        wo_sb = pp.tile([128, G, HID], bf16, tag="wo")
        nc.gpsimd.dma_start(out=wo_sb, in_=woT_d[:, :].rearrange("(h d) n -> d h n", d=128))

        # persistent activation tensors
        qrT = pp.tile([128, G, S], f32r, tag="qrT")     # rotated q, [d, h, s]
        krT = pp.tile([128, S], f32r, tag="krT")        # rotated k, [d, s]
        V_sb = pp.tile([128, KK, HD], bf16, tag="V")    # v in [s, d] per s-tile

        # ---- phase 1: projections + RoPE + V transpose ----
        with tc.tile_pool(name="p1w", bufs=1) as p1w, \
             tc.tile_pool(name="p1hs", bufs=3) as p1hs, \
             tc.tile_pool(name="rope", bufs=2) as rope, \
             tc.tile_pool(name="p1ps", bufs=1, space="PSUM") as p1ps, \
             tc.tile_pool(name="p1pt", bufs=2, space="PSUM") as p1pt:

            cos_sb = p1w.tile([128, S], f32, tag="cos")
            nc.sync.dma_start(out=cos_sb, in_=cosT_d[:, :])
            sin_sb = p1w.tile([128, S], f32, tag="sin")
            nc.sync.dma_start(out=sin_sb, in_=sinTs_d[:, :])
            wq_sb = p1w.tile([128, KK, G * HD], f32r, tag="wq")
            nc.gpsimd.dma_start(out=wq_sb, in_=wqT_d[:, :].rearrange("(kk p) f -> p kk f", p=128))
            wk_sb = p1w.tile([128, KK, HD], f32r, tag="wk")
            nc.gpsimd.dma_start(out=wk_sb, in_=wkT_d[:, :].rearrange("(kk p) f -> p kk f", p=128))
            wv_sb = p1w.tile([128, KK, HD], f32r, tag="wv")
            nc.gpsimd.dma_start(out=wv_sb, in_=wvT_d[:, :].rearrange("(kk p) f -> p kk f", p=128))

            for sc in range(4):
                s0 = sc * 512
                ps_q = [p1ps.tile([128, 512], f32, tag=f"psq{h}", name=f"psq{h}")
                        for h in range(G)]
                ps_k = p1ps.tile([128, 512], f32, tag="psk")
                ps_v = p1ps.tile([128, 512], f32, tag="psv")
                for kk in range(KK):
                    hs_t = p1hs.tile([128, 512], f32r, tag="hs")
                    nc.gpsimd.dma_start(out=hs_t, in_=hsT_d[kk * 128:(kk + 1) * 128, s0:s0 + 512])
                    first, last = kk == 0, kk == KK - 1
                    for h in range(G):
                        nc.tensor.matmul(ps_q[h], wq_sb[:, kk, h * 128:(h + 1) * 128], hs_t,
                                         start=first, stop=last)
                    nc.tensor.matmul(ps_k, wk_sb[:, kk, :], hs_t, start=first, stop=last)
                    nc.tensor.matmul(ps_v, wv_sb[:, kk, :], hs_t, start=first, stop=last)

                # RoPE on q heads and k
                for h in range(G + 1):
                    src = ps_k if h == G else ps_q[h]
                    dst = krT[:, s0:s0 + 512] if h == G else qrT[:, h, s0:s0 + 512]
                    qf = rope.tile([128, 512], f32, tag="qf")
                    nc.vector.tensor_copy(out=qf, in_=src)
                    qs = rope.tile([128, 512], f32, tag="qs")
                    nc.sync.dma_start(out=qs[0:64, :], in_=qf[64:128, :])
                    nc.sync.dma_start(out=qs[64:128, :], in_=qf[0:64, :])
                    t1 = rope.tile([128, 512], f32, tag="t1")
                    nc.vector.tensor_mul(out=t1, in0=qs, in1=sin_sb[:, s0:s0 + 512])
                    t2 = rope.tile([128, 512], f32, tag="t2")
                    nc.vector.tensor_mul(out=t2, in0=qf, in1=cos_sb[:, s0:s0 + 512])
                    nc.vector.tensor_add(out=dst, in0=t1, in1=t2)

                # V: psum [d, s] -> transpose to [s, d] tiles (bf16)
                vf = rope.tile([128, 512], f32, tag="vf")
                nc.vector.tensor_copy(out=vf, in_=ps_v)
                for i in range(4):
                    pt = p1pt.tile([128, 128], f32, tag="pt")
                    nc.tensor.transpose(pt, vf[:, i * 128:(i + 1) * 128], ident)
                    nc.vector.tensor_copy(out=V_sb[:, sc * 4 + i, :], in_=pt)

        # ---- phase 2: attention ----
        with tc.tile_pool(name="mpool", bufs=3) as mpool, \
             tc.tile_pool(name="spool", bufs=2) as spool, \
             tc.tile_pool(name="epool", bufs=2) as epool, \
             tc.tile_pool(name="apool", bufs=2) as apool, \
             tc.tile_pool(name="atpool", bufs=2) as atpool, \
             tc.tile_pool(name="otpool", bufs=2) as otpool, \
             tc.tile_pool(name="opool", bufs=3) as opool, \
             tc.tile_pool(name="stat", bufs=6) as stat, \
             tc.tile_pool(name="ps_s", bufs=4, space="PSUM") as ps_s_pool, \
             tc.tile_pool(name="ps_av", bufs=2, space="PSUM") as ps_av_pool, \
             tc.tile_pool(name="ps_o", bufs=2, space="PSUM") as ps_o_pool:

            for qc in range(NQC):
                mt = []
                for qtl in range(2):
                    qt = qc * 2 + qtl
                    m = mpool.tile([128, S], f32, tag="mask", name=f"m{qt}")
                    nc.sync.dma_start(out=m, in_=mask_d[qt * 128:(qt + 1) * 128, :])
                    mt.append(m)

                OT_sb = otpool.tile([128, G, QCW], bf16, tag="OT")
                for h in range(G):
                    at = atpool.tile([128, KK, QCW], bf16, tag="attnT")
                    for qtl in range(2):
                        qt = qc * 2 + qtl
                        sc_sb = spool.tile([128, S], f32, tag="sc")
                        for c in range(4):
                            ps = ps_s_pool.tile([128, 512], f32, tag="ps_s")
                            nc.tensor.matmul(ps, qrT[:, h, qt * 128:(qt + 1) * 128],
                                             krT[:, c * 512:(c + 1) * 512],
                                             start=True, stop=True)
                            nc.vector.scalar_tensor_tensor(
                                out=sc_sb[:, c * 512:(c + 1) * 512], in0=ps,
                                scalar=gain_sb[:, qt:qt + 1],
                                in1=mt[qtl][:, c * 512:(c + 1) * 512],
                                op0=Alu.mult, op1=Alu.add)
                        E_sb = epool.tile([128, S], f32, tag="E")
                        rowsum = stat.tile([128, 1], f32, tag="rowsum")
                        nc.scalar.activation(out=E_sb, in_=sc_sb, func=Act.Exp,
                                             accum_out=rowsum[:, 0:1])
                        rs = stat.tile([128, 1], f32, tag="rs")
                        nc.vector.reciprocal(out=rs, in_=rowsum)
                        attn_sb = apool.tile([128, S], bf16, tag="attn")
                        nc.scalar.activation(out=attn_sb, in_=E_sb, func=Act.Copy,
                                             scale=rs[:, 0:1])
                        nc.gpsimd.dma_start(out=attn_d[h, qt * 128:(qt + 1) * 128, :],
                                            in_=attn_sb)
                        nc.sync.dma_start_transpose(
                            at[:, :, qtl * 128:(qtl + 1) * 128], attn_sb)
                    # AV for head h over this q-chunk
                    ps_av = ps_av_pool.tile([128, QCW], f32, tag="ps_av")
                    for kk in range(KK):
                        nc.tensor.matmul(ps_av, V_sb[:, kk, :], at[:, kk, :],
                                         start=(kk == 0), stop=(kk == KK - 1))
                    nc.vector.tensor_copy(out=OT_sb[:, h, :], in_=ps_av)

                # o-proj for this q-chunk
                for qtl in range(2):
                    qt = qc * 2 + qtl
                    for nck in range(4):
                        ps_o = ps_o_pool.tile([128, 512], f32, tag="ps_o")
                        for h in range(G):
                            nc.tensor.matmul(ps_o, OT_sb[:, h, qtl * 128:(qtl + 1) * 128],
                                             wo_sb[:, h, nck * 512:(nck + 1) * 512],
                                             start=(h == 0), stop=(h == G - 1))
                        out_sb = opool.tile([128, 512], f32, tag="out")
                        nc.vector.tensor_copy(out=out_sb, in_=ps_o)
                        nc.sync.dma_start(
                            out=outp_d[qt * 128:(qt + 1) * 128, nck * 512:(nck + 1) * 512],
                            in_=out_sb)

    nc.finalize()
    return nc


def _get_nc():
    if "nc" not in _CACHE:
        _CACHE["nc"] = _build()
    return _CACHE["nc"]


def kernel(hidden_states, wq, wk, wv, wo, cos, sin, attention_mask, precision_bias,
           _trace=False):
    from concourse.bass_utils import run_bass_kernel_spmd

    f32 = np.float32
    hs = np.asarray(hidden_states, dtype=f32)
    wq = np.asarray(wq, dtype=f32)
    wk = np.asarray(wk, dtype=f32)
    wv = np.asarray(wv, dtype=f32)
    wo = np.asarray(wo, dtype=f32)
    cos = np.asarray(cos, dtype=f32)
    sin = np.asarray(sin, dtype=f32)
    mask = np.asarray(attention_mask, dtype=f32)
    pbias = np.asarray(precision_bias, dtype=f32)

    cosT = np.ascontiguousarray(cos[0].T)                  # [128, S]
    sinTs = np.ascontiguousarray(sin[0].T)
    sinTs[:HD // 2] *= -1.0
    mask2d = np.ascontiguousarray(np.broadcast_to(mask[0, 0], (S, S)))
    hsT = [np.ascontiguousarray(hs[b].T) for b in range(B)]
    gains = []
    for b in range(B):
        gn = ((1.0 + pbias[b]) * SCALE).astype(f32)        # [S]
        gains.append(np.ascontiguousarray(gn.reshape(S // 128, 128).T))  # [128, 16]

    in_maps = []
    for c in range(8):
        b, g = divmod(c, NKV)
        in_maps.append({
            "hsT": hsT[b],
            "wqT": np.ascontiguousarray(wq[g * G * HD:(g + 1) * G * HD].T),
            "wkT": np.ascontiguousarray(wk[g * HD:(g + 1) * HD].T),
            "wvT": np.ascontiguousarray(wv[g * HD:(g + 1) * HD].T),
            "woT": np.ascontiguousarray(wo[:, g * G * HD:(g + 1) * G * HD].T),
            "cosT": cosT,
            "sinTs": sinTs,
            "mask": mask2d,
            "gain": gains[b],
        })

    nc = _get_nc()
    res = run_bass_kernel_spmd(nc, in_maps, core_ids=list(range(8)), trace=_trace)
    if _trace:
        _CACHE["last_exec_time_ns"] = res.exec_time_ns

    out = np.zeros((B, S, HID), dtype=f32)
    attn = np.empty((B, NH, S, S), dtype=f32)
    for c in range(8):
        b, g = divmod(c, NKV)
        attn[b, g * G:(g + 1) * G] = res.results[c]["attn_out"]
        out[b] += res.results[c]["out_partial"]
    return out, attn


# revision 11
# speedup vs baseline: 1.1493x; 1.1493x over previous
"""Trainium2 Bass kernel for CCTAttention (GQA attention + RoPE + precision-bias gain).

Sharding: 8 cores = (batch b in {0,1}) x (kv-group g in {0..3}).
Core c = 4*b + g computes q-heads 4g..4g+3 and kv head g for batch b:
  - projections (q/k/v) from hs[b] in transposed [d, s] layout (f32r matmuls)
  - RoPE via partition-shift copies; gain*(1+pbias)*SCALE folded into rotated q
  - scores built in PSUM: identity-matmul preloads the additive mask, QK accumulates
  - softmax without max-subtraction (scores are O(5) bounded); exp reads PSUM
  - attn stored bf16 (host upcasts), DMA-transposed for the bf16 AV matmul
  - out_partial = attn @ v @ wo_shard.T in bf16 (host upcasts + sums partials)
"""

import numpy as np

B, S, HID = 2, 2048, 2048
NH, NKV, HD = 16, 4, 128
G = NH // NKV          # q-heads per core
SCALE = HD ** -0.5

_CACHE = {}


def _build():
    import concourse.bacc as bacc
    import concourse.tile as tile
    from concourse import mybir
    from concourse.masks import make_identity

    f32 = mybir.dt.float32
    f32r = mybir.dt.float32r
    bf16 = mybir.dt.bfloat16
    Act = mybir.ActivationFunctionType

    nc = bacc.Bacc()

    hsT_d = nc.declare_dram_parameter("hsT", [HID, S], f32, isOutput=False)
    wqT_d = nc.declare_dram_parameter("wqT", [HID, G * HD], f32, isOutput=False)
    wkT_d = nc.declare_dram_parameter("wkT", [HID, HD], f32, isOutput=False)
    wvT_d = nc.declare_dram_parameter("wvT", [HID, HD], f32, isOutput=False)
    woT_d = nc.declare_dram_parameter("woT", [G * HD, HID], bf16, isOutput=False)
    cosT_d = nc.declare_dram_parameter("cosT", [HD, S], f32, isOutput=False)
    sinTs_d = nc.declare_dram_parameter("sinTs", [HD, S], f32, isOutput=False)
    mask_d = nc.declare_dram_parameter("mask", [S, S], bf16, isOutput=False)
    gain_d = nc.declare_dram_parameter("gain", [1, S], f32, isOutput=False)
    attn_d = nc.declare_dram_parameter("attn_out", [G, S, S], bf16, isOutput=True)
    outp_d = nc.declare_dram_parameter("out_partial", [S, HID], bf16, isOutput=True)

    KK = HID // 128     # 16 contraction tiles
    NQC = 8             # q-chunks of 256
    QCW = 256

    with tile.TileContext(nc) as tc, tc.tile_pool(name="persist", bufs=1) as pp:
        ident = pp.tile([128, 128], f32, tag="ident")
        make_identity(nc, ident)
        ident_bf = pp.tile([128, 128], bf16, tag="ident_bf")
        make_identity(nc, ident_bf)
        gain_b = pp.tile([128, S], f32, tag="gain_b")
        nc.sync.dma_start(out=gain_b, in_=gain_d[:, :].to_broadcast((128, S)))
        wo_sb = pp.tile([128, G, HID], bf16, tag="wo")
        nc.sync.dma_start(out=wo_sb, in_=woT_d[:, :].rearrange("(h d) n -> d h n", d=128))

        # persistent activation tensors
        qrT = pp.tile([128, G, S], f32r, tag="qrT")     # rotated, gain-scaled q [d, h, s]
        krT = pp.tile([128, S], f32r, tag="krT")        # rotated k, [d, s]
        V_sb = pp.tile([128, KK, HD], bf16, tag="V")    # v in [s, d] per s-tile

        # ---- phase 1: projections + RoPE + V transpose ----
        with tc.tile_pool(name="p1w", bufs=1) as p1w, \
             tc.tile_pool(name="p1hs", bufs=3) as p1hs, \
             tc.tile_pool(name="rope", bufs=2) as rope, \
             tc.tile_pool(name="p1ps", bufs=1, space="PSUM") as p1ps, \
             tc.tile_pool(name="p1pt", bufs=2, space="PSUM") as p1pt:

            cos_sb = p1w.tile([128, S], f32, tag="cos")
            nc.sync.dma_start(out=cos_sb, in_=cosT_d[:, :])
            sin_sb = p1w.tile([128, S], f32, tag="sin")
            nc.sync.dma_start(out=sin_sb, in_=sinTs_d[:, :])
            wq_sb = p1w.tile([128, KK, G * HD], f32r, tag="wq")
            nc.gpsimd.dma_start(out=wq_sb, in_=wqT_d[:, :].rearrange("(kk p) f -> p kk f", p=128))
            wk_sb = p1w.tile([128, KK, HD], f32r, tag="wk")
            nc.gpsimd.dma_start(out=wk_sb, in_=wkT_d[:, :].rearrange("(kk p) f -> p kk f", p=128))
            wv_sb = p1w.tile([128, KK, HD], f32r, tag="wv")
            nc.gpsimd.dma_start(out=wv_sb, in_=wvT_d[:, :].rearrange("(kk p) f -> p kk f", p=128))

            for sc in range(4):
                s0 = sc * 512
                ps_q = [p1ps.tile([128, 512], f32, tag=f"psq{h}", name=f"psq{h}")
                        for h in range(G)]
                ps_k = p1ps.tile([128, 512], f32, tag="psk")
                ps_v = p1ps.tile([128, 512], f32, tag="psv")
                for kk in range(KK):
                    hs_t = p1hs.tile([128, 512], f32r, tag="hs")
                    nc.gpsimd.dma_start(out=hs_t, in_=hsT_d[kk * 128:(kk + 1) * 128, s0:s0 + 512])
                    first, last = kk == 0, kk == KK - 1
                    for h in range(G):
                        nc.tensor.matmul(ps_q[h], wq_sb[:, kk, h * 128:(h + 1) * 128], hs_t,
                                         start=first, stop=last)
                    nc.tensor.matmul(ps_k, wk_sb[:, kk, :], hs_t, start=first, stop=last)
                    nc.tensor.matmul(ps_v, wv_sb[:, kk, :], hs_t, start=first, stop=last)

                # RoPE on q heads and k; gain*SCALE folded into q
                for h in range(G + 1):
                    src = ps_k if h == G else ps_q[h]
                    dst = krT[:, s0:s0 + 512] if h == G else qrT[:, h, s0:s0 + 512]
                    qf = rope.tile([128, 512], f32, tag="qf")
                    nc.vector.tensor_copy(out=qf, in_=src)
                    qs = rope.tile([128, 512], f32, tag="qs")
                    nc.sync.dma_start(out=qs[0:64, :], in_=qf[64:128, :])
                    nc.sync.dma_start(out=qs[64:128, :], in_=qf[0:64, :])
                    t1 = rope.tile([128, 512], f32, tag="t1")
                    nc.vector.tensor_mul(out=t1, in0=qs, in1=sin_sb[:, s0:s0 + 512])
                    t2 = rope.tile([128, 512], f32, tag="t2")
                    nc.vector.tensor_mul(out=t2, in0=qf, in1=cos_sb[:, s0:s0 + 512])
                    if h == G:
                        nc.vector.tensor_add(out=dst, in0=t1, in1=t2)
                    else:
                        t3 = rope.tile([128, 512], f32, tag="t3")
                        nc.vector.tensor_add(out=t3, in0=t1, in1=t2)
                        nc.vector.tensor_mul(out=dst, in0=t3, in1=gain_b[:, s0:s0 + 512])

                # V: psum [d, s] -> transpose to [s, d] tiles (bf16)
                vf = rope.tile([128, 512], f32, tag="vf")
                nc.vector.tensor_copy(out=vf, in_=ps_v)
                for i in range(4):
                    pt = p1pt.tile([128, 128], f32, tag="pt")
                    nc.tensor.transpose(pt, vf[:, i * 128:(i + 1) * 128], ident)
                    nc.vector.tensor_copy(out=V_sb[:, sc * 4 + i, :], in_=pt)

        # ---- phase 2: attention ----
        with tc.tile_pool(name="mpool", bufs=5) as mpool, \
             tc.tile_pool(name="epool", bufs=3) as epool, \
             tc.tile_pool(name="apool", bufs=3) as apool, \
             tc.tile_pool(name="atpool", bufs=2) as atpool, \
             tc.tile_pool(name="otpool", bufs=2) as otpool, \
             tc.tile_pool(name="opool", bufs=3) as opool, \
             tc.tile_pool(name="stat", bufs=8) as stat, \
             tc.tile_pool(name="ps_s", bufs=4, space="PSUM") as ps_s_pool, \
             tc.tile_pool(name="ps_av", bufs=2, space="PSUM") as ps_av_pool, \
             tc.tile_pool(name="ps_o", bufs=2, space="PSUM") as ps_o_pool:

            for qc in range(NQC):
                mt = []
                for qtl in range(2):
                    qt = qc * 2 + qtl
                    m = mpool.tile([128, S], bf16, tag="mask", name=f"m{qt}")
                    nc.sync.dma_start(out=m, in_=mask_d[qt * 128:(qt + 1) * 128, :])
                    mt.append(m)

                OT_sb = otpool.tile([128, G, QCW], bf16, tag="OT")
                for h in range(G):
                    at = atpool.tile([128, KK, QCW], bf16, tag="attnT")
                    for qtl in range(2):
                        qt = qc * 2 + qtl
                        E_sb = epool.tile([128, S], f32, tag="E")
                        acc = [stat.tile([128, 1], f32, tag=f"acc{c}", name=f"acc{c}")
                               for c in range(4)]
                        for c in range(4):
                            ps = ps_s_pool.tile([128, 512], f32, tag="ps_s")
                            nc.tensor.matmul(ps, ident_bf,
                                             mt[qtl][:, c * 512:(c + 1) * 512],
                                             start=True, stop=False)
                            nc.tensor.matmul(ps, qrT[:, h, qt * 128:(qt + 1) * 128],
                                             krT[:, c * 512:(c + 1) * 512],
                                             start=False, stop=True)
                            nc.scalar.activation(out=E_sb[:, c * 512:(c + 1) * 512],
                                                 in_=ps, func=Act.Exp,
                                                 accum_out=acc[c][:, 0:1])
                        s01 = stat.tile([128, 1], f32, tag="s01")
                        nc.vector.tensor_add(out=s01, in0=acc[0], in1=acc[1])
                        s23 = stat.tile([128, 1], f32, tag="s23")
                        nc.vector.tensor_add(out=s23, in0=acc[2], in1=acc[3])
                        tot = stat.tile([128, 1], f32, tag="tot")
                        nc.vector.tensor_add(out=tot, in0=s01, in1=s23)
                        rs = stat.tile([128, 1], f32, tag="rs")
                        nc.vector.reciprocal(out=rs, in_=tot)
                        attn_sb = apool.tile([128, S], bf16, tag="attn")
                        nc.scalar.activation(out=attn_sb, in_=E_sb, func=Act.Copy,
                                             scale=rs[:, 0:1])
                        nc.sync.dma_start(out=attn_d[h, qt * 128:(qt + 1) * 128, :],
                                          in_=attn_sb)
                        nc.sync.dma_start_transpose(
                            at[:, :, qtl * 128:(qtl + 1) * 128], attn_sb)
                    # AV for head h over this q-chunk
                    ps_av = ps_av_pool.tile([128, QCW], f32, tag="ps_av")
                    for kk in range(KK):
                        nc.tensor.matmul(ps_av, V_sb[:, kk, :], at[:, kk, :],
                                         start=(kk == 0), stop=(kk == KK - 1))
                    nc.vector.tensor_copy(out=OT_sb[:, h, :], in_=ps_av)

                # o-proj for this q-chunk
                for qtl in range(2):
                    qt = qc * 2 + qtl
                    for nck in range(4):
                        ps_o = ps_o_pool.tile([128, 512], f32, tag="ps_o")
                        for h in range(G):
                            nc.tensor.matmul(ps_o, OT_sb[:, h, qtl * 128:(qtl + 1) * 128],
                                             wo_sb[:, h, nck * 512:(nck + 1) * 512],
                                             start=(h == 0), stop=(h == G - 1))
                        out_sb = opool.tile([128, 512], bf16, tag="out")
                        nc.vector.tensor_copy(out=out_sb, in_=ps_o)
                        nc.sync.dma_start(
                            out=outp_d[qt * 128:(qt + 1) * 128, nck * 512:(nck + 1) * 512],
                            in_=out_sb)

    nc.finalize()
    return nc


def _get_nc():
    if "nc" not in _CACHE:
        _CACHE["nc"] = _build()
    return _CACHE["nc"]


def kernel(hidden_states, wq, wk, wv, wo, cos, sin, attention_mask, precision_bias,
           _trace=False):
    import ml_dtypes
    from concourse.bass_utils import run_bass_kernel_spmd

    f32 = np.float32
    bf16 = ml_dtypes.bfloat16
    hs = np.asarray(hidden_states, dtype=f32)
    wq = np.asarray(wq, dtype=f32)
    wk = np.asarray(wk, dtype=f32)
    wv = np.asarray(wv, dtype=f32)
    wo = np.asarray(wo, dtype=f32)
    cos = np.asarray(cos, dtype=f32)
    sin = np.asarray(sin, dtype=f32)
    mask = np.asarray(attention_mask, dtype=f32)
    pbias = np.asarray(precision_bias, dtype=f32)

    cosT = np.ascontiguousarray(cos[0].T)                  # [128, S]
    sinTs = np.ascontiguousarray(sin[0].T)
    sinTs[:HD // 2] *= -1.0
    # clamp so the bf16 cast cannot overflow to -inf (exp underflows to 0 far
    # above this threshold, so the clamp never changes the math)
    mask2d = np.maximum(np.broadcast_to(mask[0, 0], (S, S)), -1.0e30).astype(bf16)
    hsT = [np.ascontiguousarray(hs[b].T) for b in range(B)]
    gains = [((1.0 + pbias[b]) * SCALE).astype(f32).reshape(1, S) for b in range(B)]

    in_maps = []
    for c in range(8):
        b, g = divmod(c, NKV)
        in_maps.append({
            "hsT": hsT[b],
            "wqT": np.ascontiguousarray(wq[g * G * HD:(g + 1) * G * HD].T),
            "wkT": np.ascontiguousarray(wk[g * HD:(g + 1) * HD].T),
            "wvT": np.ascontiguousarray(wv[g * HD:(g + 1) * HD].T),
            "woT": np.ascontiguousarray(wo[:, g * G * HD:(g + 1) * G * HD].T).astype(bf16),
            "cosT": cosT,
            "sinTs": sinTs,
            "mask": mask2d,
            "gain": gains[b],
        })

    nc = _get_nc()
    res = run_bass_kernel_spmd(nc, in_maps, core_ids=list(range(8)), trace=_trace)
    if _trace:
        _CACHE["last_exec_time_ns"] = res.exec_time_ns

    out = np.zeros((B, S, HID), dtype=f32)
    attn = np.empty((B, NH, S, S), dtype=f32)
    for c in range(8):
        b, g = divmod(c, NKV)
        attn[b, g * G:(g + 1) * G] = res.results[c]["attn_out"].astype(f32)
        out[b] += res.results[c]["out_partial"].astype(f32)
    return out, attn


# revision 13
# speedup vs baseline: 1.1526x; 1.0028x over previous
"""Trainium2 Bass kernel for CCTAttention (GQA attention + RoPE + precision-bias gain).

Sharding: 8 cores = (batch b in {0,1}) x (kv-group g in {0..3}).
Core c = 4*b + g computes q-heads 4g..4g+3 and kv head g for batch b:
  - projections (q/k/v) from hs[b] in transposed [d, s] layout (f32r matmuls)
  - RoPE via partition-shift copies; gain*(1+pbias)*SCALE folded into rotated q
  - scores built in PSUM: identity-matmul preloads the additive mask, QK accumulates
  - softmax without max-subtraction (scores are O(5) bounded); exp reads PSUM
  - attn stored bf16 (host upcasts), DMA-transposed for the bf16 AV matmul
  - out_partial = attn @ v @ wo_shard.T in bf16 (host upcasts + sums partials)
"""

import numpy as np

B, S, HID = 2, 2048, 2048
NH, NKV, HD = 16, 4, 128
G = NH // NKV          # q-heads per core
SCALE = HD ** -0.5

_CACHE = {}


def _build():
    import concourse.bacc as bacc
    import concourse.tile as tile
    from concourse import mybir
    from concourse.masks import make_identity

    f32 = mybir.dt.float32
    f32r = mybir.dt.float32r
    bf16 = mybir.dt.bfloat16
    Act = mybir.ActivationFunctionType

    nc = bacc.Bacc()

    hsT_d = nc.declare_dram_parameter("hsT", [HID, S], f32, isOutput=False)
    wqT_d = nc.declare_dram_parameter("wqT", [HID, G * HD], f32, isOutput=False)
    wkT_d = nc.declare_dram_parameter("wkT", [HID, HD], f32, isOutput=False)
    wvT_d = nc.declare_dram_parameter("wvT", [HID, HD], f32, isOutput=False)
    woT_d = nc.declare_dram_parameter("woT", [G * HD, HID], bf16, isOutput=False)
    cosT_d = nc.declare_dram_parameter("cosT", [HD, S], f32, isOutput=False)
    sinTs_d = nc.declare_dram_parameter("sinTs", [HD, S], f32, isOutput=False)
    mask_d = nc.declare_dram_parameter("mask", [S, S], bf16, isOutput=False)
    gain_d = nc.declare_dram_parameter("gain", [1, S], f32, isOutput=False)
    attn_d = nc.declare_dram_parameter("attn_out", [G, S, S], bf16, isOutput=True)
    outp_d = nc.declare_dram_parameter("out_partial", [S, HID], bf16, isOutput=True)

    KK = HID // 128     # 16 contraction tiles
    NQC = 8             # q-chunks of 256
    QCW = 256

    with tile.TileContext(nc) as tc, tc.tile_pool(name="persist", bufs=1) as pp:
        ident = pp.tile([128, 128], f32, tag="ident")
        make_identity(nc, ident)
        ident_bf = pp.tile([128, 128], bf16, tag="ident_bf")
        make_identity(nc, ident_bf)
        gain_b = pp.tile([128, S], f32, tag="gain_b")
        nc.sync.dma_start(out=gain_b, in_=gain_d[:, :].to_broadcast((128, S)))
        wo_sb = pp.tile([128, G, HID], bf16, tag="wo")
        nc.sync.dma_start(out=wo_sb, in_=woT_d[:, :].rearrange("(h d) n -> d h n", d=128))

        # persistent activation tensors
        qrT = pp.tile([128, G, S], f32r, tag="qrT")     # rotated, gain-scaled q [d, h, s]
        krT = pp.tile([128, S], f32r, tag="krT")        # rotated k, [d, s]
        V_sb = pp.tile([128, KK, HD], bf16, tag="V")    # v in [s, d] per s-tile

        # ---- phase 1: projections + RoPE + V transpose ----
        with tc.tile_pool(name="p1w", bufs=1) as p1w, \
             tc.tile_pool(name="p1hs", bufs=3) as p1hs, \
             tc.tile_pool(name="rope", bufs=2) as rope, \
             tc.tile_pool(name="p1ps", bufs=1, space="PSUM") as p1ps, \
             tc.tile_pool(name="p1pt", bufs=2, space="PSUM") as p1pt:

            cos_sb = p1w.tile([128, S], f32, tag="cos")
            nc.sync.dma_start(out=cos_sb, in_=cosT_d[:, :])
            sin_sb = p1w.tile([128, S], f32, tag="sin")
            nc.sync.dma_start(out=sin_sb, in_=sinTs_d[:, :])
            wq_sb = p1w.tile([128, KK, G * HD], f32r, tag="wq")
            nc.gpsimd.dma_start(out=wq_sb, in_=wqT_d[:, :].rearrange("(kk p) f -> p kk f", p=128))
            wk_sb = p1w.tile([128, KK, HD], f32r, tag="wk")
            nc.gpsimd.dma_start(out=wk_sb, in_=wkT_d[:, :].rearrange("(kk p) f -> p kk f", p=128))
            wv_sb = p1w.tile([128, KK, HD], f32r, tag="wv")
            nc.gpsimd.dma_start(out=wv_sb, in_=wvT_d[:, :].rearrange("(kk p) f -> p kk f", p=128))

            for sc in range(4):
                s0 = sc * 512
                ps_q = [p1ps.tile([128, 512], f32, tag=f"psq{h}", name=f"psq{h}")
                        for h in range(G)]
                ps_k = p1ps.tile([128, 512], f32, tag="psk")
                ps_v = p1ps.tile([128, 512], f32, tag="psv")
                for kk in range(KK):
                    hs_t = p1hs.tile([128, 512], f32r, tag="hs")
                    nc.gpsimd.dma_start(out=hs_t, in_=hsT_d[kk * 128:(kk + 1) * 128, s0:s0 + 512])
                    first, last = kk == 0, kk == KK - 1
                    for h in range(G):
                        nc.tensor.matmul(ps_q[h], wq_sb[:, kk, h * 128:(h + 1) * 128], hs_t,
                                         start=first, stop=last)
                    nc.tensor.matmul(ps_k, wk_sb[:, kk, :], hs_t, start=first, stop=last)
                    nc.tensor.matmul(ps_v, wv_sb[:, kk, :], hs_t, start=first, stop=last)

                # RoPE on q heads and k; gain*SCALE folded into q
                for h in range(G + 1):
                    src = ps_k if h == G else ps_q[h]
                    dst = krT[:, s0:s0 + 512] if h == G else qrT[:, h, s0:s0 + 512]
                    qf = rope.tile([128, 512], f32, tag="qf")
                    nc.vector.tensor_copy(out=qf, in_=src)
                    qs = rope.tile([128, 512], f32, tag="qs")
                    nc.sync.dma_start(out=qs[0:64, :], in_=qf[64:128, :])
                    nc.sync.dma_start(out=qs[64:128, :], in_=qf[0:64, :])
                    t1 = rope.tile([128, 512], f32, tag="t1")
                    nc.vector.tensor_mul(out=t1, in0=qs, in1=sin_sb[:, s0:s0 + 512])
                    t2 = rope.tile([128, 512], f32, tag="t2")
                    nc.vector.tensor_mul(out=t2, in0=qf, in1=cos_sb[:, s0:s0 + 512])
                    if h == G:
                        nc.vector.tensor_add(out=dst, in0=t1, in1=t2)
                    else:
                        t3 = rope.tile([128, 512], f32, tag="t3")
                        nc.vector.tensor_add(out=t3, in0=t1, in1=t2)
                        nc.vector.tensor_mul(out=dst, in0=t3, in1=gain_b[:, s0:s0 + 512])

                # V: psum [d, s] -> transpose to [s, d] tiles (bf16)
                vf = rope.tile([128, 512], f32, tag="vf")
                nc.vector.tensor_copy(out=vf, in_=ps_v)
                for i in range(4):
                    pt = p1pt.tile([128, 128], f32, tag="pt")
                    nc.tensor.transpose(pt, vf[:, i * 128:(i + 1) * 128], ident)
                    nc.vector.tensor_copy(out=V_sb[:, sc * 4 + i, :], in_=pt)

        # ---- phase 2: attention ----
        with tc.tile_pool(name="mpool", bufs=5) as mpool, \
             tc.tile_pool(name="epool", bufs=3) as epool, \
             tc.tile_pool(name="apool", bufs=3) as apool, \
             tc.tile_pool(name="atpool", bufs=5) as atpool, \
             tc.tile_pool(name="otpool", bufs=2) as otpool, \
             tc.tile_pool(name="opool", bufs=3) as opool, \
             tc.tile_pool(name="stat", bufs=8) as stat, \
             tc.tile_pool(name="ps_s", bufs=4, space="PSUM") as ps_s_pool, \
             tc.tile_pool(name="ps_av", bufs=2, space="PSUM") as ps_av_pool, \
             tc.tile_pool(name="ps_o", bufs=2, space="PSUM") as ps_o_pool:

            for qc in range(NQC):
                mt = []
                for qtl in range(2):
                    qt = qc * 2 + qtl
                    m = mpool.tile([128, S], bf16, tag="mask", name=f"m{qt}")
                    nc.sync.dma_start(out=m, in_=mask_d[qt * 128:(qt + 1) * 128, :])
                    mt.append(m)

                OT_sb = otpool.tile([128, G, QCW], bf16, tag="OT")
                ats = [atpool.tile([128, KK, QCW], bf16, tag="attnT", name=f"at{h}")
                       for h in range(G)]
                for h in range(G):
                    at = ats[h]
                    for qtl in range(2):
                        qt = qc * 2 + qtl
                        E_sb = epool.tile([128, S], f32, tag="E")
                        acc = [stat.tile([128, 1], f32, tag=f"acc{c}", name=f"acc{c}")
                               for c in range(4)]
                        for c in range(4):
                            ps = ps_s_pool.tile([128, 512], f32, tag="ps_s")
                            nc.tensor.matmul(ps, ident_bf,
                                             mt[qtl][:, c * 512:(c + 1) * 512],
                                             start=True, stop=False)
                            nc.tensor.matmul(ps, qrT[:, h, qt * 128:(qt + 1) * 128],
                                             krT[:, c * 512:(c + 1) * 512],
                                             start=False, stop=True)
                            nc.scalar.activation(out=E_sb[:, c * 512:(c + 1) * 512],
                                                 in_=ps, func=Act.Exp,
                                                 accum_out=acc[c][:, 0:1])
                        s01 = stat.tile([128, 1], f32, tag="s01")
                        nc.vector.tensor_add(out=s01, in0=acc[0], in1=acc[1])
                        s23 = stat.tile([128, 1], f32, tag="s23")
                        nc.vector.tensor_add(out=s23, in0=acc[2], in1=acc[3])
                        tot = stat.tile([128, 1], f32, tag="tot")
                        nc.vector.tensor_add(out=tot, in0=s01, in1=s23)
                        rs = stat.tile([128, 1], f32, tag="rs")
                        nc.vector.reciprocal(out=rs, in_=tot)
                        attn_sb = apool.tile([128, S], bf16, tag="attn")
                        if (h + qtl) % 2 == 0:
                            nc.scalar.activation(out=attn_sb, in_=E_sb, func=Act.Copy,
                                                 scale=rs[:, 0:1])
                        else:
                            nc.vector.tensor_scalar_mul(out=attn_sb, in0=E_sb,
                                                        scalar1=rs[:, 0:1])
                        nc.sync.dma_start(out=attn_d[h, qt * 128:(qt + 1) * 128, :],
                                          in_=attn_sb)
                        nc.sync.dma_start_transpose(
                            at[:, :, qtl * 128:(qtl + 1) * 128], attn_sb)
                # AV after all heads' scores are in flight
                for h in range(G):
                    ps_av = ps_av_pool.tile([128, QCW], f32, tag="ps_av")
                    for kk in range(KK):
                        nc.tensor.matmul(ps_av, V_sb[:, kk, :], ats[h][:, kk, :],
                                         start=(kk == 0), stop=(kk == KK - 1))
                    nc.vector.tensor_copy(out=OT_sb[:, h, :], in_=ps_av)

                # o-proj for this q-chunk
                for qtl in range(2):
                    qt = qc * 2 + qtl
                    for nck in range(4):
                        ps_o = ps_o_pool.tile([128, 512], f32, tag="ps_o")
                        for h in range(G):
                            nc.tensor.matmul(ps_o, OT_sb[:, h, qtl * 128:(qtl + 1) * 128],
                                             wo_sb[:, h, nck * 512:(nck + 1) * 512],
                                             start=(h == 0), stop=(h == G - 1))
                        out_sb = opool.tile([128, 512], bf16, tag="out")
                        nc.vector.tensor_copy(out=out_sb, in_=ps_o)
                        nc.sync.dma_start(
                            out=outp_d[qt * 128:(qt + 1) * 128, nck * 512:(nck + 1) * 512],
                            in_=out_sb)

    nc.finalize()
    return nc


def _get_nc():
    if "nc" not in _CACHE:
        _CACHE["nc"] = _build()
    return _CACHE["nc"]


def kernel(hidden_states, wq, wk, wv, wo, cos, sin, attention_mask, precision_bias,
           _trace=False):
    import ml_dtypes
    from concourse.bass_utils import run_bass_kernel_spmd

    f32 = np.float32
    bf16 = ml_dtypes.bfloat16
    hs = np.asarray(hidden_states, dtype=f32)
    wq = np.asarray(wq, dtype=f32)
    wk = np.asarray(wk, dtype=f32)
    wv = np.asarray(wv, dtype=f32)
    wo = np.asarray(wo, dtype=f32)
    cos = np.asarray(cos, dtype=f32)
    sin = np.asarray(sin, dtype=f32)
    mask = np.asarray(attention_mask, dtype=f32)
    pbias = np.asarray(precision_bias, dtype=f32)

    cosT = np.ascontiguousarray(cos[0].T)                  # [128, S]
    sinTs = np.ascontiguousarray(sin[0].T)
    sinTs[:HD // 2] *= -1.0
    # clamp so the bf16 cast cannot overflow to -inf (exp underflows to 0 far
    # above this threshold, so the clamp never changes the math)
    mask2d = np.maximum(np.broadcast_to(mask[0, 0], (S, S)), -1.0e30).astype(bf16)
    hsT = [np.ascontiguousarray(hs[b].T) for b in range(B)]
    gains = [((1.0 + pbias[b]) * SCALE).astype(f32).reshape(1, S) for b in range(B)]

    in_maps = []
    for c in range(8):
        b, g = divmod(c, NKV)
        in_maps.append({
            "hsT": hsT[b],
            "wqT": np.ascontiguousarray(wq[g * G * HD:(g + 1) * G * HD].T),
            "wkT": np.ascontiguousarray(wk[g * HD:(g + 1) * HD].T),
            "wvT": np.ascontiguousarray(wv[g * HD:(g + 1) * HD].T),
            "woT": np.ascontiguousarray(wo[:, g * G * HD:(g + 1) * G * HD].T).astype(bf16),
            "cosT": cosT,
            "sinTs": sinTs,
            "mask": mask2d,
            "gain": gains[b],
        })

    nc = _get_nc()
    res = run_bass_kernel_spmd(nc, in_maps, core_ids=list(range(8)), trace=_trace)
    if _trace:
        _CACHE["last_exec_time_ns"] = res.exec_time_ns

    out = np.zeros((B, S, HID), dtype=f32)
    attn = np.empty((B, NH, S, S), dtype=f32)
    for c in range(8):
        b, g = divmod(c, NKV)
        attn[b, g * G:(g + 1) * G] = res.results[c]["attn_out"].astype(f32)
        out[b] += res.results[c]["out_partial"].astype(f32)
    return out, attn


# revision 14
# speedup vs baseline: 1.1984x; 1.0397x over previous
"""Trainium2 Bass kernel for CCTAttention (GQA attention + RoPE + precision-bias gain).

Sharding: 8 cores = (batch b in {0,1}) x (kv-group g in {0..3}).
Core c = 4*b + g computes q-heads 4g..4g+3 and kv head g for batch b:
  - projections (q/k/v) from hs[b] in transposed [d, s] layout (f32r matmuls)
  - RoPE via partition-shift copies; gain*(1+pbias)*SCALE folded into rotated q
  - scores built in PSUM: identity-matmul preloads the additive mask, QK accumulates
  - softmax without max-subtraction (scores are O(5) bounded); exp reads PSUM
  - attn stored bf16 (host upcasts), DMA-transposed for the bf16 AV matmul
  - out_partial = attn @ v @ wo_shard.T in bf16 (host upcasts + sums partials)
"""

import numpy as np

B, S, HID = 2, 2048, 2048
NH, NKV, HD = 16, 4, 128
G = NH // NKV          # q-heads per core
SCALE = HD ** -0.5

_CACHE = {}


def _build():
    import concourse.bacc as bacc
    import concourse.tile as tile
    from concourse import mybir
    from concourse.masks import make_identity

    f32 = mybir.dt.float32
    f32r = mybir.dt.float32r
    bf16 = mybir.dt.bfloat16
    Act = mybir.ActivationFunctionType

    nc = bacc.Bacc()

    hsT_d = nc.declare_dram_parameter("hsT", [HID, S], f32, isOutput=False)
    wqT_d = nc.declare_dram_parameter("wqT", [HID, G * HD], f32, isOutput=False)
    wkT_d = nc.declare_dram_parameter("wkT", [HID, HD], f32, isOutput=False)
    wvT_d = nc.declare_dram_parameter("wvT", [HID, HD], f32, isOutput=False)
    woT_d = nc.declare_dram_parameter("woT", [G * HD, HID], bf16, isOutput=False)
    cosT_d = nc.declare_dram_parameter("cosT", [HD, S], f32, isOutput=False)
    sinTs_d = nc.declare_dram_parameter("sinTs", [HD, S], f32, isOutput=False)
    mask_d = nc.declare_dram_parameter("mask", [S, S], bf16, isOutput=False)
    gain_d = nc.declare_dram_parameter("gain", [1, S], f32, isOutput=False)
    attn_d = nc.declare_dram_parameter("attn_out", [G, S, S], bf16, isOutput=True)
    outp_d = nc.declare_dram_parameter("out_partial", [S, HID], bf16, isOutput=True)

    KK = HID // 128     # 16 contraction tiles
    NQC = 8             # q-chunks of 256
    QCW = 256

    with tile.TileContext(nc) as tc, tc.tile_pool(name="persist", bufs=1) as pp:
        ident = pp.tile([128, 128], f32, tag="ident")
        make_identity(nc, ident)
        ident_bf = pp.tile([128, 128], bf16, tag="ident_bf")
        make_identity(nc, ident_bf)
        gain_b = pp.tile([128, S], f32, tag="gain_b")
        nc.sync.dma_start(out=gain_b, in_=gain_d[:, :].to_broadcast((128, S)))
        wo_sb = pp.tile([128, G, HID], bf16, tag="wo")
        nc.sync.dma_start(out=wo_sb, in_=woT_d[:, :].rearrange("(h d) n -> d h n", d=128))

        # persistent activation tensors
        qrT = pp.tile([128, G, S], f32r, tag="qrT")     # rotated, gain-scaled q [d, h, s]
        krT = pp.tile([128, S], f32r, tag="krT")        # rotated k, [d, s]
        V_sb = pp.tile([128, KK, HD], bf16, tag="V")    # v in [s, d] per s-tile

        # ---- phase 1: projections + RoPE + V transpose ----
        with tc.tile_pool(name="p1w", bufs=1) as p1w, \
             tc.tile_pool(name="p1hs", bufs=3) as p1hs, \
             tc.tile_pool(name="rope", bufs=2) as rope, \
             tc.tile_pool(name="p1ps", bufs=1, space="PSUM") as p1ps, \
             tc.tile_pool(name="p1pt", bufs=2, space="PSUM") as p1pt:

            cos_sb = p1w.tile([128, S], f32, tag="cos")
            nc.sync.dma_start(out=cos_sb, in_=cosT_d[:, :])
            sin_sb = p1w.tile([128, S], f32, tag="sin")
            nc.sync.dma_start(out=sin_sb, in_=sinTs_d[:, :])
            wq_sb = p1w.tile([128, KK, G * HD], f32r, tag="wq")
            nc.gpsimd.dma_start(out=wq_sb, in_=wqT_d[:, :].rearrange("(kk p) f -> p kk f", p=128))
            wk_sb = p1w.tile([128, KK, HD], f32r, tag="wk")
            nc.gpsimd.dma_start(out=wk_sb, in_=wkT_d[:, :].rearrange("(kk p) f -> p kk f", p=128))
            wv_sb = p1w.tile([128, KK, HD], f32r, tag="wv")
            nc.gpsimd.dma_start(out=wv_sb, in_=wvT_d[:, :].rearrange("(kk p) f -> p kk f", p=128))

            for sc in range(4):
                s0 = sc * 512
                ps_q = [p1ps.tile([128, 512], f32, tag=f"psq{h}", name=f"psq{h}")
                        for h in range(G)]
                ps_k = p1ps.tile([128, 512], f32, tag="psk")
                ps_v = p1ps.tile([128, 512], f32, tag="psv")
                for kk in range(KK):
                    hs_t = p1hs.tile([128, 512], f32r, tag="hs")
                    nc.gpsimd.dma_start(out=hs_t, in_=hsT_d[kk * 128:(kk + 1) * 128, s0:s0 + 512])
                    first, last = kk == 0, kk == KK - 1
                    for h in range(G):
                        nc.tensor.matmul(ps_q[h], wq_sb[:, kk, h * 128:(h + 1) * 128], hs_t,
                                         start=first, stop=last)
                    nc.tensor.matmul(ps_k, wk_sb[:, kk, :], hs_t, start=first, stop=last)
                    nc.tensor.matmul(ps_v, wv_sb[:, kk, :], hs_t, start=first, stop=last)

                # RoPE on q heads and k; gain*SCALE folded into q
                for h in range(G + 1):
                    src = ps_k if h == G else ps_q[h]
                    dst = krT[:, s0:s0 + 512] if h == G else qrT[:, h, s0:s0 + 512]
                    qf = rope.tile([128, 512], f32, tag="qf")
                    nc.vector.tensor_copy(out=qf, in_=src)
                    qs = rope.tile([128, 512], f32, tag="qs")
                    nc.sync.dma_start(out=qs[0:64, :], in_=qf[64:128, :])
                    nc.sync.dma_start(out=qs[64:128, :], in_=qf[0:64, :])
                    t1 = rope.tile([128, 512], f32, tag="t1")
                    nc.vector.tensor_mul(out=t1, in0=qs, in1=sin_sb[:, s0:s0 + 512])
                    t2 = rope.tile([128, 512], f32, tag="t2")
                    nc.vector.tensor_mul(out=t2, in0=qf, in1=cos_sb[:, s0:s0 + 512])
                    if h == G:
                        nc.vector.tensor_add(out=dst, in0=t1, in1=t2)
                    else:
                        t3 = rope.tile([128, 512], f32, tag="t3")
                        nc.vector.tensor_add(out=t3, in0=t1, in1=t2)
                        nc.vector.tensor_mul(out=dst, in0=t3, in1=gain_b[:, s0:s0 + 512])

                # V: psum [d, s] -> transpose to [s, d] tiles (bf16)
                vf = rope.tile([128, 512], f32, tag="vf")
                nc.vector.tensor_copy(out=vf, in_=ps_v)
                for i in range(4):
                    pt = p1pt.tile([128, 128], f32, tag="pt")
                    nc.tensor.transpose(pt, vf[:, i * 128:(i + 1) * 128], ident)
                    nc.vector.tensor_copy(out=V_sb[:, sc * 4 + i, :], in_=pt)

        # ---- phase 2: attention ----
        with tc.tile_pool(name="mpool", bufs=5) as mpool, \
             tc.tile_pool(name="epool", bufs=3) as epool, \
             tc.tile_pool(name="apool", bufs=3) as apool, \
             tc.tile_pool(name="atpool", bufs=5) as atpool, \
             tc.tile_pool(name="otpool", bufs=2) as otpool, \
             tc.tile_pool(name="opool", bufs=3) as opool, \
             tc.tile_pool(name="stat", bufs=8) as stat, \
             tc.tile_pool(name="ps_s", bufs=2, space="PSUM") as ps_s_pool, \
             tc.tile_pool(name="ps_av", bufs=2, space="PSUM") as ps_av_pool, \
             tc.tile_pool(name="ps_o", bufs=1, space="PSUM") as ps_o_pool:

            def o_proj(qc, OT_sb):
                for qtl in range(2):
                    qt = qc * 2 + qtl
                    out_row = opool.tile([128, S], bf16, tag="out", name=f"o{qt}")
                    for nck in range(2):
                        ps_o = ps_o_pool.tile([128, 1024], f32, tag="ps_o")
                        for half in range(2):
                            for h in range(G):
                                nc.tensor.matmul(
                                    ps_o[:, half * 512:(half + 1) * 512],
                                    OT_sb[:, h, qtl * 128:(qtl + 1) * 128],
                                    wo_sb[:, h, (2 * nck + half) * 512:(2 * nck + half + 1) * 512],
                                    start=(h == 0), stop=(h == G - 1))
                        nc.vector.tensor_copy(
                            out=out_row[:, nck * 1024:(nck + 1) * 1024], in_=ps_o)
                    nc.sync.dma_start(out=outp_d[qt * 128:(qt + 1) * 128, :], in_=out_row)

            prev = None
            for qc in range(NQC):
                mt = []
                for qtl in range(2):
                    qt = qc * 2 + qtl
                    m = mpool.tile([128, S], bf16, tag="mask", name=f"m{qt}")
                    nc.sync.dma_start(out=m, in_=mask_d[qt * 128:(qt + 1) * 128, :])
                    mt.append(m)

                OT_sb = otpool.tile([128, G, QCW], bf16, tag="OT")
                ats = [atpool.tile([128, KK, QCW], bf16, tag="attnT", name=f"at{h}")
                       for h in range(G)]
                for h in range(G):
                    at = ats[h]
                    for qtl in range(2):
                        qt = qc * 2 + qtl
                        E_sb = epool.tile([128, S], f32, tag="E")
                        acc = [stat.tile([128, 1], f32, tag=f"acc{c}", name=f"acc{c}")
                               for c in range(2)]
                        for c in range(2):
                            ps = ps_s_pool.tile([128, 1024], f32, tag="ps_s")
                            for half in range(2):
                                col = c * 1024 + half * 512
                                sl = ps[:, half * 512:(half + 1) * 512]
                                nc.tensor.matmul(sl, ident_bf,
                                                 mt[qtl][:, col:col + 512],
                                                 start=True, stop=False)
                                nc.tensor.matmul(sl, qrT[:, h, qt * 128:(qt + 1) * 128],
                                                 krT[:, col:col + 512],
                                                 start=False, stop=True)
                            nc.scalar.activation(out=E_sb[:, c * 1024:(c + 1) * 1024],
                                                 in_=ps, func=Act.Exp,
                                                 accum_out=acc[c][:, 0:1])
                        tot = stat.tile([128, 1], f32, tag="tot")
                        nc.vector.tensor_add(out=tot, in0=acc[0], in1=acc[1])
                        rs = stat.tile([128, 1], f32, tag="rs")
                        nc.vector.reciprocal(out=rs, in_=tot)
                        attn_sb = apool.tile([128, S], bf16, tag="attn")
                        if (h + qtl) % 2 == 0:
                            nc.scalar.activation(out=attn_sb, in_=E_sb, func=Act.Copy,
                                                 scale=rs[:, 0:1])
                        else:
                            nc.vector.tensor_scalar_mul(out=attn_sb, in0=E_sb,
                                                        scalar1=rs[:, 0:1])
                        nc.sync.dma_start(out=attn_d[h, qt * 128:(qt + 1) * 128, :],
                                          in_=attn_sb)
                        nc.sync.dma_start_transpose(
                            at[:, :, qtl * 128:(qtl + 1) * 128], attn_sb)
                # AV after all heads' scores are in flight
                for h in range(G):
                    ps_av = ps_av_pool.tile([128, QCW], f32, tag="ps_av")
                    for kk in range(KK):
                        nc.tensor.matmul(ps_av, V_sb[:, kk, :], ats[h][:, kk, :],
                                         start=(kk == 0), stop=(kk == KK - 1))
                    nc.vector.tensor_copy(out=OT_sb[:, h, :], in_=ps_av)

                # o-proj pipelined one q-chunk behind
                if prev is not None:
                    o_proj(qc - 1, prev)
                prev = OT_sb
            o_proj(NQC - 1, prev)

    nc.finalize()
    return nc


def _get_nc():
    if "nc" not in _CACHE:
        _CACHE["nc"] = _build()
    return _CACHE["nc"]


def kernel(hidden_states, wq, wk, wv, wo, cos, sin, attention_mask, precision_bias,
           _trace=False):
    import ml_dtypes
    from concourse.bass_utils import run_bass_kernel_spmd

    f32 = np.float32
    bf16 = ml_dtypes.bfloat16
    hs = np.asarray(hidden_states, dtype=f32)
    wq = np.asarray(wq, dtype=f32)
    wk = np.asarray(wk, dtype=f32)
    wv = np.asarray(wv, dtype=f32)
    wo = np.asarray(wo, dtype=f32)
    cos = np.asarray(cos, dtype=f32)
    sin = np.asarray(sin, dtype=f32)
    mask = np.asarray(attention_mask, dtype=f32)
    pbias = np.asarray(precision_bias, dtype=f32)

    cosT = np.ascontiguousarray(cos[0].T)                  # [128, S]
    sinTs = np.ascontiguousarray(sin[0].T)
    sinTs[:HD // 2] *= -1.0
    # clamp so the bf16 cast cannot overflow to -inf (exp underflows to 0 far
    # above this threshold, so the clamp never changes the math)
    mask2d = np.maximum(np.broadcast_to(mask[0, 0], (S, S)), -1.0e30).astype(bf16)
    hsT = [np.ascontiguousarray(hs[b].T) for b in range(B)]
    gains = [((1.0 + pbias[b]) * SCALE).astype(f32).reshape(1, S) for b in range(B)]

    in_maps = []
    for c in range(8):
        b, g = divmod(c, NKV)
        in_maps.append({
            "hsT": hsT[b],
            "wqT": np.ascontiguousarray(wq[g * G * HD:(g + 1) * G * HD].T),
            "wkT": np.ascontiguousarray(wk[g * HD:(g + 1) * HD].T),
            "wvT": np.ascontiguousarray(wv[g * HD:(g + 1) * HD].T),
            "woT": np.ascontiguousarray(wo[:, g * G * HD:(g + 1) * G * HD].T).astype(bf16),
            "cosT": cosT,
            "sinTs": sinTs,
            "mask": mask2d,
            "gain": gains[b],
        })

    nc = _get_nc()
    res = run_bass_kernel_spmd(nc, in_maps, core_ids=list(range(8)), trace=_trace)
    if _trace:
        _CACHE["last_exec_time_ns"] = res.exec_time_ns

    out = np.zeros((B, S, HID), dtype=f32)
    attn = np.empty((B, NH, S, S), dtype=f32)
    for c in range(8):
        b, g = divmod(c, NKV)
        attn[b, g * G:(g + 1) * G] = res.results[c]["attn_out"].astype(f32)
        out[b] += res.results[c]["out_partial"].astype(f32)
    return out, attn
